# revision 24
# baseline (speedup 1.0000x reference)
"""Trainium2 Bass kernel for nn_GaussianSelfAttention (B=64, S=197, D=768).

Math: the reference's softmax is over a singleton axis, so attn == 1.0 exactly
and out = concat([ones(B,1,D), G @ x @ Wv + wsum*bv], axis=1) where G is the
per-image (196,197) bilinear 4-point weight matrix built from Gaussian-sampled
keys. q/k projections are dead code.

Device strategy (8 cores, data-parallel over batch, 8 images/core):
  - host builds G^T per image (tiny index math) and uploads bf16 tiles
  - gather matmul per image: sxT[d,p] = x[s,d]^T-contracted with G^T[s,p]
    (lhsT = x s-chunks, rhs = G^T s-chunks, accumulate over the 2 s-chunks)
  - projection: svT[dout,q] = Wv^T @ sxT with Wv 128x128 blocks stationary,
    moving operand = sxT in 392-col chunks (2 images), accumulated over d
  - psum -> sbuf copies alternate ScalarE/VectorE, bf16 out, chunked DMA out
  - all matmul operands bf16 (tolerance 2e-2; bf16 contributes ~3e-3)
"""

import numpy as np

import concourse.bass as bass
import concourse.mybir as mybir
import concourse.tile as tile
from concourse import bacc, bass_utils

B, S, D, P = 64, 197, 768, 196
N_CORES = 8
BPC = B // N_CORES            # images per core
Q = BPC * P                   # 1568 sampled rows per core
GRID = 14.0
NCH = 392                     # projection moving-operand chunk (2 images)
S0, S1 = 128, 69              # s-chunks of S=197

F32 = mybir.dt.float32
BF16 = mybir.dt.bfloat16

_NC = {}
_RUNNER = {}


IW = 2 * (D + P)              # 1928 cols per full-mode packed image tile
KC = 64                       # compact-mode gather contraction rows per image
PW = D + 2 * P                # 1160 cols per compact pair tile (block-diag G)


def _emit(nc, iters=1, compact=True, unroll=1):
    # compact mode: per image PAIR tile [128, 1160]: cols [0,768) the two
    # images' compacted x rows stacked (A rows 0-63, B rows 64-127), cols
    # [768,1160) block-diagonal compacted G^T ([gA;0] then [0;gB]) so one
    # K=128 matmul computes both images' gathers side by side.
    # full mode: per image [128, 1928]: cols [0,768) x s-chunk0; [768,964)
    # G^T s-chunk0; [964,1732) x s-chunk1 (rows 0-68); [1732,1928) G^T
    # s-chunk1 (rows 0-68).
    if compact:
        xg_d = nc.dram_tensor("xg0", (128, (BPC // 2) * PW), BF16,
                              kind="ExternalInput")
    else:
        xg_d = nc.dram_tensor("xg0", (128, BPC * IW), BF16,
                              kind="ExternalInput")
    wv_d = nc.dram_tensor("wv0", (128, 6 * D), BF16, kind="ExternalInput")
    o_d = nc.dram_tensor("o0", (128, 6 * Q), BF16, kind="ExternalOutput")

    with tile.TileContext(nc) as tc:
        with (
            tc.tile_pool(name="wvp", bufs=2) as wvp,
            tc.tile_pool(name="xgp", bufs=2) as xgp,
            tc.tile_pool(name="sxp", bufs=2) as sxp,
            tc.tile_pool(name="obp", bufs=2) as obp,
            tc.tile_pool(name="pg", bufs=4, space="PSUM") as pg,
            tc.tile_pool(name="pp", bufs=1, space="PSUM") as pp,
        ):
            eng_ctr = [0]

            def copy_eng():
                eng_ctr[0] += 1
                return (nc.scalar.copy if eng_ctr[0] % 2 else
                        nc.vector.tensor_copy)

            def body():
                xg = {}
                nload = BPC // 2 if compact else BPC
                iw = PW if compact else IW

                def load_img(b):
                    xg[b] = xgp.tile([128, iw], BF16, name=f"xg{b}",
                                     tag=f"xg{b}")
                    nc.sync.dma_start(
                        out=xg[b][:], in_=xg_d[:, b * iw:(b + 1) * iw])

                load_img(0)
                wvt = wvp.tile([128, 6 * D], BF16, name="wvt", tag="wvt")
                nc.sync.dma_start(out=wvt[:], in_=wv_d[:])
                for b in range(1, nload):
                    load_img(b)

                sxT = [sxp.tile([128, Q], BF16, name=f"sx{k}", tag=f"sx{k}")
                       for k in range(6)]
                osb = obp.tile([128, 6 * Q], BF16, name="osb", tag="osb")

                def gather_pair(pr):
                    # two images per matmul via block-diagonal G^T
                    t = xg[pr]
                    for mj in range(6):
                        ps = pg.tile([128, 2 * P], F32, name="psg", tag="psg")
                        nc.tensor.matmul(
                            ps[:], lhsT=t[:, mj * 128:(mj + 1) * 128],
                            rhs=t[:, D:PW], start=True, stop=True)
                        copy_eng()(out=sxT[mj][:, pr * 2 * P:(pr + 1) * 2 * P],
                                   in_=ps[:])

                def gather_full(b):
                    t = xg[b]
                    for mj in range(6):
                        ps = pg.tile([128, P], F32, name="psg", tag="psg")
                        nc.tensor.matmul(
                            ps[:], lhsT=t[:, mj * 128:(mj + 1) * 128],
                            rhs=t[:, D:D + P], start=True, stop=False)
                        nc.tensor.matmul(
                            ps[:],
                            lhsT=t[0:S1, D + P + mj * 128:
                                   D + P + (mj + 1) * 128],
                            rhs=t[0:S1, 2 * D + P:IW],
                            start=False, stop=True)
                        copy_eng()(out=sxT[mj][:, b * P:(b + 1) * P],
                                   in_=ps[:])

                for b in range(nload):
                    if compact:
                        gather_pair(b)
                    else:
                        gather_full(b)

                # projection: Wv block stationary, 4 n-chunks inner
                for m in range(6):
                    pbs = [pp.tile([128, NCH], F32, name=f"psp{n}",
                                   tag=f"psp{n}") for n in range(4)]
                    for k in range(6):
                        for n in range(4):
                            nc.tensor.matmul(
                                pbs[n][:],
                                lhsT=wvt[:, k * D + m * 128:
                                         k * D + (m + 1) * 128],
                                rhs=sxT[k][:, n * NCH:(n + 1) * NCH],
                                start=(k == 0), stop=(k == 5))
                    for n in range(4):
                        copy_eng()(out=osb[:, (m * 4 + n) * NCH:
                                           (m * 4 + n + 1) * NCH],
                                   in_=pbs[n][:])
                    nc.sync.dma_start(
                        out=o_d[:, m * 4 * NCH:(m + 1) * 4 * NCH],
                        in_=osb[:, m * 4 * NCH:(m + 1) * 4 * NCH])

            if iters == 1:
                body()
            else:
                assert iters % unroll == 0
                with tc.For_i(0, iters // unroll, 1):
                    for _ in range(unroll):
                        body()


def _build(iters=1, compact=True, unroll=1):
    key = (iters, compact, unroll)
    if key not in _NC:
        nc = bacc.Bacc("TRN2", target_bir_lowering=False, debug=False,
                       num_devices=N_CORES)
        _emit(nc, iters, compact, unroll)
        nc.compile()
        _NC[key] = nc
    return _NC[key]


def _keys_weights(img_ids, avgs, std_devs, noise):
    ids = np.asarray(img_ids).astype(np.int64)
    a = np.asarray(avgs, np.float32)[ids]
    sd = np.asarray(std_devs, np.float32)[ids]
    nz = np.asarray(noise, np.float32)
    kx = (nz[:, 0] - a[:, 0]) / sd[:, 0]
    ky = (nz[:, 1] - a[:, 1]) / sd[:, 1]
    x1, x2 = np.ceil(kx), np.floor(kx)
    y1, y2 = np.ceil(ky), np.floor(ky)

    def bilin(px, py):
        return (1.0 - np.abs(px - kx)) * (1.0 - np.abs(py - ky))

    def idx(px, py):
        return (GRID * py + px).astype(np.int32) % S

    combos = [(bilin(x1, y1), idx(x1, y1)), (bilin(x2, y1), idx(x2, y1)),
              (bilin(x1, y2), idx(x1, y2)), (bilin(x2, y2), idx(x2, y2))]
    return combos


def _pack_inputs(x, img_ids, Wv, avgs, std_devs, noise):
    bfloat16 = mybir.dt.np(BF16)
    x = np.asarray(x, np.float32)
    wv = np.asarray(Wv, np.float32)
    wvp = np.ascontiguousarray(
        wv.reshape(6, 128, D).transpose(1, 0, 2).reshape(128, 6 * D)
    ).astype(bfloat16)

    combos = _keys_weights(img_ids, avgs, std_devs, noise)

    # can every image's gather be compacted to <= KC distinct rows?
    pt = np.arange(P)
    rowsets = [np.unique(np.concatenate([ii[b] for _, ii in combos]))
               for b in range(B)]
    compact = max(len(r) for r in rowsets) <= KC

    if not compact:
        # full G^T (S x P) per image from the 4 bilinear scatter points
        gT = np.zeros((B, S, P), np.float32)
        ib = np.repeat(np.arange(B), P)
        ip = np.tile(np.arange(P), B)
        for ww, ii in combos:
            np.add.at(gT, (ib, ii.ravel(), ip), ww.ravel())
        gT = gT.astype(bfloat16)

    xbf = x.astype(bfloat16)
    in_maps = []
    for c in range(N_CORES):
        if compact:
            xp = np.zeros((128, (BPC // 2) * PW), bfloat16)
            for b in range(BPC):
                img = c * BPC + b
                rows = rowsets[img]
                r = len(rows)
                g = np.zeros((KC, P), np.float32)
                for ww, ii in combos:
                    np.add.at(g, (np.searchsorted(rows, ii[img]), pt),
                              ww[img])
                p0 = (b % 2) * KC
                o0 = (b // 2) * PW
                g0 = o0 + D + (b % 2) * P       # block-diagonal G^T slot
                xp[p0:p0 + r, o0:o0 + D] = xbf[img, rows]
                xp[p0:p0 + KC, g0:g0 + P] = g.astype(bfloat16)
        else:
            xp = np.zeros((128, BPC * IW), bfloat16)
            for b in range(BPC):
                img = c * BPC + b
                o0 = b * IW
                xp[:, o0:o0 + D] = xbf[img, 0:128]
                xp[:, o0 + D:o0 + D + P] = gT[img, 0:128]
                xp[0:S1, o0 + D + P:o0 + 2 * D + P] = xbf[img, 128:S]
                xp[0:S1, o0 + 2 * D + P:o0 + IW] = gT[img, 128:S]
        in_maps.append({"xg0": xp, "wv0": wvp})
    return in_maps, compact


def _unpack_out(o_np):
    # o_np: (128, 6*4*392) bf16, m-major then n (svT chunks) -> (BPC, S, D)
    o = np.asarray(o_np, np.float32).reshape(128, 6, 4, NCH)
    sv = o.transpose(2, 3, 1, 0).reshape(Q, D)
    out = np.ones((BPC, S, D), np.float32)
    out[:, 1:, :] = sv.reshape(BPC, P, D)
    return out


def _get_runner(iters=1, compact=True, unroll=1):
    """Build the sharded PJRT callable once and cache it."""
    key = (iters, compact, unroll)
    if key in _RUNNER:
        return _RUNNER[key]
    import jax
    from jax.experimental.shard_map import shard_map
    from jax.sharding import Mesh, PartitionSpec
    from concourse import bass2jax, mybir as _mybir

    nc = _build(iters, compact, unroll)
    bass2jax.install_neuronx_cc_hook()
    in_names, out_names, out_avals, zero_outs = [], [], [], []
    part_name = (nc.partition_id_tensor.name
                 if nc.partition_id_tensor else None)
    for alloc in nc.m.functions[0].allocations:
        if not isinstance(alloc, _mybir.MemoryLocationSet):
            continue
        name = alloc.memorylocations[0].name
        if alloc.kind == "ExternalInput":
            if name != part_name:
                in_names.append(name)
        elif alloc.kind == "ExternalOutput":
            shape = tuple(alloc.tensor_shape)
            dtype = _mybir.dt.np(alloc.dtype)
            out_names.append(name)
            out_avals.append(jax.core.ShapedArray(shape, dtype))
            zero_outs.append(np.zeros(shape, dtype))
    n_params = len(in_names)
    all_names = in_names + out_names
    if part_name is not None:
        all_names = all_names + [part_name]
    donate = tuple(range(n_params, n_params + len(out_names)))

    def _body(*args):
        operands = list(args)
        if part_name is not None:
            operands.append(bass2jax.partition_id_tensor())
        outs = bass2jax._bass_exec_p.bind(
            *operands,
            out_avals=tuple(out_avals),
            in_names=tuple(all_names),
            out_names=tuple(out_names),
            lowering_input_output_aliases=(),
            sim_require_finite=True,
            sim_require_nnan=True,
            nc=nc,
        )
        return tuple(outs)

    devices = jax.devices()[:N_CORES]
    mesh = Mesh(np.asarray(devices), ("core",))
    specs = (PartitionSpec("core"),) * (n_params + len(out_names))
    fn = jax.jit(
        shard_map(_body, mesh=mesh, in_specs=specs,
                  out_specs=(PartitionSpec("core"),) * len(out_names),
                  check_rep=False),
        donate_argnums=donate, keep_unused=True)

    def run(in_maps):
        concat_in = [
            np.concatenate([np.asarray(m[nm]) for m in in_maps], axis=0)
            for nm in in_names
        ]
        concat_zero = [
            np.zeros((N_CORES * z.shape[0], *z.shape[1:]), z.dtype)
            for z in zero_outs
        ]
        arrs = fn(*concat_in, *concat_zero)
        return [
            {nm: np.asarray(arrs[i]).reshape(N_CORES, *out_avals[i].shape)[c]
             for i, nm in enumerate(out_names)}
            for c in range(N_CORES)
        ]

    _RUNNER[key] = run
    return run


class _Res:
    def __init__(self, results):
        self.results = results
        self.exec_time_ns = None


def run_cores(in_maps, trace=False, iters=1, compact=True):
    return _Res(_get_runner(iters, compact)(in_maps))


def kernel(x, img_ids, mask=None, Wq=None, bq=None, Wk=None, bk=None,
           Wv=None, bv=None, avgs=None, std_devs=None, noise=None,
           _trace=False, _results=None):
    in_maps, compact = _pack_inputs(x, img_ids, Wv, avgs, std_devs, noise)
    res = run_cores(in_maps, trace=_trace, compact=compact)
    if _results is not None:
        _results.append(res)
    out = np.concatenate(
        [_unpack_out(res.results[c]["o0"]) for c in range(N_CORES)], axis=0)
    bv_np = np.asarray(bv, np.float32) if bv is not None else None
    if bv_np is not None and np.any(bv_np):
        # sample() is affine: add (sum_i w_i) * bv for the sampled rows.
        wsum = sum(ww for ww, _ in
                   _keys_weights(img_ids, avgs, std_devs, noise))
        out[:, 1:, :] += wsum[:, :, None] * bv_np[None, None, :]
    return out


# revision 29
# speedup vs baseline: 1.3912x; 1.3912x over previous
"""Trainium2 Bass kernel for nn_GaussianSelfAttention (B=64, S=197, D=768).

Math: the reference's softmax is over a singleton axis, so attn == 1.0 exactly
and out = concat([ones(B,1,D), G @ x @ Wv + wsum*bv], axis=1) where G is the
per-image (196,197) bilinear 4-point weight matrix built from Gaussian-sampled
keys. q/k projections are dead code.

Device strategy (8 cores, data-parallel over batch, 8 images/core):
  - host builds G^T per image (tiny index math) and uploads bf16 tiles
  - gather matmul per image: sxT[d,p] = x[s,d]^T-contracted with G^T[s,p]
    (lhsT = x s-chunks, rhs = G^T s-chunks, accumulate over the 2 s-chunks)
  - projection: svT[dout,q] = Wv^T @ sxT with Wv 128x128 blocks stationary,
    moving operand = sxT in 392-col chunks (2 images), accumulated over d
  - psum -> sbuf copies alternate ScalarE/VectorE, bf16 out, chunked DMA out
  - all matmul operands bf16 (tolerance 2e-2; bf16 contributes ~3e-3)
"""

import numpy as np

import concourse.bass as bass
import concourse.mybir as mybir
import concourse.tile as tile
from concourse import bacc, bass_utils

B, S, D, P = 64, 197, 768, 196
N_CORES = 8
BPC = B // N_CORES            # images per core
Q = BPC * P                   # 1568 sampled rows per core
GRID = 14.0
NCH = 392                     # projection moving-operand chunk (2 images)
S0, S1 = 128, 69              # s-chunks of S=197

F32 = mybir.dt.float32
BF16 = mybir.dt.bfloat16

_NC = {}
_RUNNER = {}


IW = 2 * (D + P)              # 1928 cols per full-mode packed image tile
KC = 64                       # compact-mode gather contraction rows per image
PW = D + 2 * P                # 1160 cols per compact pair tile (block-diag G)


def _emit(nc, iters=1, compact=True, unroll=1):
    # compact mode: per image PAIR tile [128, 1160]: cols [0,768) the two
    # images' compacted x rows stacked (A rows 0-63, B rows 64-127), cols
    # [768,1160) block-diagonal compacted G^T ([gA;0] then [0;gB]) so one
    # K=128 matmul computes both images' gathers side by side.
    # full mode: per image [128, 1928]: cols [0,768) x s-chunk0; [768,964)
    # G^T s-chunk0; [964,1732) x s-chunk1 (rows 0-68); [1732,1928) G^T
    # s-chunk1 (rows 0-68).
    if compact:
        xg_d = nc.dram_tensor("xg0", (128, (BPC // 2) * PW), BF16,
                              kind="ExternalInput")
    else:
        xg_d = nc.dram_tensor("xg0", (128, BPC * IW), BF16,
                              kind="ExternalInput")
    wv_d = nc.dram_tensor("wv0", (128, 6 * D), BF16, kind="ExternalInput")
    o_d = nc.dram_tensor("o0", (128, 6 * Q), BF16, kind="ExternalOutput")

    with tile.TileContext(nc) as tc:
        with (
            tc.tile_pool(name="wvp", bufs=2) as wvp,
            tc.tile_pool(name="xgp", bufs=2) as xgp,
            tc.tile_pool(name="sxp", bufs=2) as sxp,
            tc.tile_pool(name="obp", bufs=2) as obp,
            tc.tile_pool(name="pg", bufs=3, space="PSUM") as pg,
            tc.tile_pool(name="pp", bufs=1, space="PSUM") as pp,
        ):
            eng_ctr = [0]

            def copy_eng():
                eng_ctr[0] += 1
                return (nc.scalar.copy if eng_ctr[0] % 2 else
                        nc.vector.tensor_copy)

            def body():
                xg = {}
                nload = BPC // 2 if compact else BPC
                iw = PW if compact else IW
                mrg = 2 if compact else 1       # pair-tiles per input DMA

                def load_img(b):
                    xg[b] = xgp.tile([128, mrg * iw], BF16, name=f"xg{b}",
                                     tag=f"xg{b}")
                    nc.sync.dma_start(
                        out=xg[b][:],
                        in_=xg_d[:, b * mrg * iw:(b + 1) * mrg * iw])

                load_img(0)
                wvt = wvp.tile([128, 6 * D], BF16, name="wvt", tag="wvt")
                nc.sync.dma_start(out=wvt[:], in_=wv_d[:])
                for b in range(1, nload // mrg):
                    load_img(b)

                sxT = [sxp.tile([128, Q], BF16, name=f"sx{k}", tag=f"sx{k}")
                       for k in range(6)]
                osb = obp.tile([128, 6 * Q], BF16, name="osb", tag="osb")

                def gather_pair(pr):
                    # two images per matmul via block-diagonal G^T
                    t = xg[pr // 2]
                    c0 = (pr % 2) * PW
                    for mj in range(6):
                        ps = pg.tile([128, 2 * P], F32, name="psg", tag="psg")
                        nc.tensor.matmul(
                            ps[:], lhsT=t[:, c0 + mj * 128:
                                          c0 + (mj + 1) * 128],
                            rhs=t[:, c0 + D:c0 + PW], start=True, stop=True)
                        copy_eng()(out=sxT[mj][:, pr * 2 * P:(pr + 1) * 2 * P],
                                   in_=ps[:])

                def gather_full(b):
                    t = xg[b]
                    for mj in range(6):
                        ps = pg.tile([128, P], F32, name="psg", tag="psg")
                        nc.tensor.matmul(
                            ps[:], lhsT=t[:, mj * 128:(mj + 1) * 128],
                            rhs=t[:, D:D + P], start=True, stop=False)
                        nc.tensor.matmul(
                            ps[:],
                            lhsT=t[0:S1, D + P + mj * 128:
                                   D + P + (mj + 1) * 128],
                            rhs=t[0:S1, 2 * D + P:IW],
                            start=False, stop=True)
                        copy_eng()(out=sxT[mj][:, b * P:(b + 1) * P],
                                   in_=ps[:])

                for b in range(nload):
                    if compact:
                        gather_pair(b)
                    else:
                        gather_full(b)

                # projection: Wv block stationary, 4 n-chunks inner
                for m in range(6):
                    pbs = [pp.tile([128, NCH], F32, name=f"psp{n}",
                                   tag=f"psp{n}") for n in range(4)]
                    for k in range(6):
                        for n in range(4):
                            nc.tensor.matmul(
                                pbs[n][:],
                                lhsT=wvt[:, k * D + m * 128:
                                         k * D + (m + 1) * 128],
                                rhs=sxT[k][:, n * NCH:(n + 1) * NCH],
                                start=(k == 0), stop=(k == 5))
                    for n in range(4):
                        copy_eng()(out=osb[:, (m * 4 + n) * NCH:
                                           (m * 4 + n + 1) * NCH],
                                   in_=pbs[n][:])
                    if m % 2 == 1:
                        # out DMA off the sync queue so next iteration's
                        # input DMAs are not blocked behind it
                        nc.scalar.dma_start(
                            out=o_d[:, (m - 1) * 4 * NCH:(m + 1) * 4 * NCH],
                            in_=osb[:, (m - 1) * 4 * NCH:(m + 1) * 4 * NCH])

            if iters == 1:
                body()
            else:
                assert iters % unroll == 0
                with tc.For_i(0, iters // unroll, 1):
                    for _ in range(unroll):
                        body()


def _build(iters=1, compact=True, unroll=1):
    key = (iters, compact, unroll)
    if key not in _NC:
        nc = bacc.Bacc("TRN2", target_bir_lowering=False, debug=False,
                       num_devices=N_CORES)
        _emit(nc, iters, compact, unroll)
        nc.compile()
        _NC[key] = nc
    return _NC[key]


def _keys_weights(img_ids, avgs, std_devs, noise):
    ids = np.asarray(img_ids).astype(np.int64)
    a = np.asarray(avgs, np.float32)[ids]
    sd = np.asarray(std_devs, np.float32)[ids]
    nz = np.asarray(noise, np.float32)
    kx = (nz[:, 0] - a[:, 0]) / sd[:, 0]
    ky = (nz[:, 1] - a[:, 1]) / sd[:, 1]
    x1, x2 = np.ceil(kx), np.floor(kx)
    y1, y2 = np.ceil(ky), np.floor(ky)

    def bilin(px, py):
        return (1.0 - np.abs(px - kx)) * (1.0 - np.abs(py - ky))

    def idx(px, py):
        return (GRID * py + px).astype(np.int32) % S

    combos = [(bilin(x1, y1), idx(x1, y1)), (bilin(x2, y1), idx(x2, y1)),
              (bilin(x1, y2), idx(x1, y2)), (bilin(x2, y2), idx(x2, y2))]
    return combos


def _pack_inputs(x, img_ids, Wv, avgs, std_devs, noise):
    bfloat16 = mybir.dt.np(BF16)
    x = np.asarray(x, np.float32)
    wv = np.asarray(Wv, np.float32)
    wvp = np.ascontiguousarray(
        wv.reshape(6, 128, D).transpose(1, 0, 2).reshape(128, 6 * D)
    ).astype(bfloat16)

    combos = _keys_weights(img_ids, avgs, std_devs, noise)

    # can every image's gather be compacted to <= KC distinct rows?
    pt = np.arange(P)
    rowsets = [np.unique(np.concatenate([ii[b] for _, ii in combos]))
               for b in range(B)]
    compact = max(len(r) for r in rowsets) <= KC

    if not compact:
        # full G^T (S x P) per image from the 4 bilinear scatter points
        gT = np.zeros((B, S, P), np.float32)
        ib = np.repeat(np.arange(B), P)
        ip = np.tile(np.arange(P), B)
        for ww, ii in combos:
            np.add.at(gT, (ib, ii.ravel(), ip), ww.ravel())
        gT = gT.astype(bfloat16)

    xbf = x.astype(bfloat16)
    in_maps = []
    for c in range(N_CORES):
        if compact:
            xp = np.zeros((128, (BPC // 2) * PW), bfloat16)
            for b in range(BPC):
                img = c * BPC + b
                rows = rowsets[img]
                r = len(rows)
                g = np.zeros((KC, P), np.float32)
                for ww, ii in combos:
                    np.add.at(g, (np.searchsorted(rows, ii[img]), pt),
                              ww[img])
                p0 = (b % 2) * KC
                o0 = (b // 2) * PW
                g0 = o0 + D + (b % 2) * P       # block-diagonal G^T slot
                xp[p0:p0 + r, o0:o0 + D] = xbf[img, rows]
                xp[p0:p0 + KC, g0:g0 + P] = g.astype(bfloat16)
        else:
            xp = np.zeros((128, BPC * IW), bfloat16)
            for b in range(BPC):
                img = c * BPC + b
                o0 = b * IW
                xp[:, o0:o0 + D] = xbf[img, 0:128]
                xp[:, o0 + D:o0 + D + P] = gT[img, 0:128]
                xp[0:S1, o0 + D + P:o0 + 2 * D + P] = xbf[img, 128:S]
                xp[0:S1, o0 + 2 * D + P:o0 + IW] = gT[img, 128:S]
        in_maps.append({"xg0": xp, "wv0": wvp})
    return in_maps, compact


def _unpack_out(o_np):
    # o_np: (128, 6*4*392) bf16, m-major then n (svT chunks) -> (BPC, S, D)
    o = np.asarray(o_np, np.float32).reshape(128, 6, 4, NCH)
    sv = o.transpose(2, 3, 1, 0).reshape(Q, D)
    out = np.ones((BPC, S, D), np.float32)
    out[:, 1:, :] = sv.reshape(BPC, P, D)
    return out


def _get_runner(iters=1, compact=True, unroll=1):
    """Build the sharded PJRT callable once and cache it."""
    key = (iters, compact, unroll)
    if key in _RUNNER:
        return _RUNNER[key]
    import jax
    from jax.experimental.shard_map import shard_map
    from jax.sharding import Mesh, PartitionSpec
    from concourse import bass2jax, mybir as _mybir

    nc = _build(iters, compact, unroll)
    bass2jax.install_neuronx_cc_hook()
    in_names, out_names, out_avals, zero_outs = [], [], [], []
    part_name = (nc.partition_id_tensor.name
                 if nc.partition_id_tensor else None)
    for alloc in nc.m.functions[0].allocations:
        if not isinstance(alloc, _mybir.MemoryLocationSet):
            continue
        name = alloc.memorylocations[0].name
        if alloc.kind == "ExternalInput":
            if name != part_name:
                in_names.append(name)
        elif alloc.kind == "ExternalOutput":
            shape = tuple(alloc.tensor_shape)
            dtype = _mybir.dt.np(alloc.dtype)
            out_names.append(name)
            out_avals.append(jax.core.ShapedArray(shape, dtype))
            zero_outs.append(np.zeros(shape, dtype))
    n_params = len(in_names)
    all_names = in_names + out_names
    if part_name is not None:
        all_names = all_names + [part_name]
    donate = tuple(range(n_params, n_params + len(out_names)))

    def _body(*args):
        operands = list(args)
        if part_name is not None:
            operands.append(bass2jax.partition_id_tensor())
        outs = bass2jax._bass_exec_p.bind(
            *operands,
            out_avals=tuple(out_avals),
            in_names=tuple(all_names),
            out_names=tuple(out_names),
            lowering_input_output_aliases=(),
            sim_require_finite=True,
            sim_require_nnan=True,
            nc=nc,
        )
        return tuple(outs)

    devices = jax.devices()[:N_CORES]
    mesh = Mesh(np.asarray(devices), ("core",))
    specs = (PartitionSpec("core"),) * (n_params + len(out_names))
    fn = jax.jit(
        shard_map(_body, mesh=mesh, in_specs=specs,
                  out_specs=(PartitionSpec("core"),) * len(out_names),
                  check_rep=False),
        donate_argnums=donate, keep_unused=True)

    def run(in_maps):
        concat_in = [
            np.concatenate([np.asarray(m[nm]) for m in in_maps], axis=0)
            for nm in in_names
        ]
        concat_zero = [
            np.zeros((N_CORES * z.shape[0], *z.shape[1:]), z.dtype)
            for z in zero_outs
        ]
        arrs = fn(*concat_in, *concat_zero)
        return [
            {nm: np.asarray(arrs[i]).reshape(N_CORES, *out_avals[i].shape)[c]
             for i, nm in enumerate(out_names)}
            for c in range(N_CORES)
        ]

    _RUNNER[key] = run
    return run


class _Res:
    def __init__(self, results):
        self.results = results
        self.exec_time_ns = None


def run_cores(in_maps, trace=False, iters=1, compact=True):
    return _Res(_get_runner(iters, compact)(in_maps))


def kernel(x, img_ids, mask=None, Wq=None, bq=None, Wk=None, bk=None,
           Wv=None, bv=None, avgs=None, std_devs=None, noise=None,
           _trace=False, _results=None):
    in_maps, compact = _pack_inputs(x, img_ids, Wv, avgs, std_devs, noise)
    res = run_cores(in_maps, trace=_trace, compact=compact)
    if _results is not None:
        _results.append(res)
    out = np.concatenate(
        [_unpack_out(res.results[c]["o0"]) for c in range(N_CORES)], axis=0)
    bv_np = np.asarray(bv, np.float32) if bv is not None else None
    if bv_np is not None and np.any(bv_np):
        # sample() is affine: add (sum_i w_i) * bv for the sampled rows.
        wsum = sum(ww for ww, _ in
                   _keys_weights(img_ids, avgs, std_devs, noise))
        out[:, 1:, :] += wsum[:, :, None] * bv_np[None, None, :]
    return out
